# revision 1
# baseline (speedup 1.0000x reference)
"""Trainium2 Bass kernel for CausalSelfAttention with kron-structured bias and
column-masked causal attention.

Shapes (hardcoded): x (4,1024,512), H=8 heads, HD=64, attn_bias (8,64,64)
expanded by kron(ones(8,8)) onto the top-left 512x512 of the (1024,1024)
score matrix. Causal tril mask with every 16th column (j%16==15) zeroed.

Sharding: 8 cores = 4 batches x 2 head-groups (4 heads each). Every core runs
an identical program on its own slice:
  - computes Q^T,K^T (feature-major) and V (token-major, with fused bias and a
    ones column per head for the softmax denominator) for its 4 heads,
  - scores S^T = K^T.T @ Q^T per (head, 128-row key tile) with the kron bias
    folded in as a rank-64 second matmul (E^T, repeat(attn_bias)) accumulating
    into the same PSUM tile,
  - P^T = exp(S^T + colmask_bias) on ScalarE (column mask via per-partition
    bias of -1e30), triangular mask on the single diagonal-crossing 128-col
    block via a 0/1 multiply,
  - O^T_ext = V_ext.T @ P^T accumulated over key tiles (row 64 = softmax
    denominator), normalized via reciprocal + ones-broadcast matmul,
  - partial projection Z = y^T.T @ Wp^T slice. Host sums the two per-batch
    partials and adds bp.

All matmuls run as float32r (4-byte fp32 data, fast PE streaming mode).
"""

import os
import sys
import types

import numpy as np

import concourse.bass as bass
import concourse.bacc as bacc
import concourse.tile as tile
from concourse import mybir
from concourse.bass_utils import run_bass_kernel_spmd


def _ensure_axon_hooks():
    """bass_utils' trace path imports antenv.axon_hooks unconditionally; some
    images lack that module. Provide it (and register the real NTFF hook when
    the axon boot shim is available) so tracing degrades gracefully."""
    try:
        import antenv.axon_hooks  # noqa: F401
        return
    except ImportError:
        pass
    m = types.ModuleType("antenv.axon_hooks")
    m._hook = None
    m.set_axon_ntff_profile_hook = lambda h: setattr(m, "_hook", h)
    m.get_axon_ntff_profile_hook = lambda: m._hook
    sys.modules["antenv.axon_hooks"] = m
    try:
        import antenv
        antenv.axon_hooks = m
    except ImportError:
        pass
    try:
        from trn_agent_boot.trn_boot import _ntff_profile_via_ctypes
        m.set_axon_ntff_profile_hook(
            _ntff_profile_via_ctypes("/opt/axon/libaxon_pjrt.so")
        )
    except Exception:
        pass


_ensure_axon_hooks()

F32 = mybir.dt.float32
F32R = mybir.dt.float32r
AFT = mybir.ActivationFunctionType

B, T, C, H = 4, 1024, 512, 8
HD = 64
SCALE = 1.0 / 8.0
GH = 4          # heads per core
N_CORES = 8

_CACHE = {}
LAST_RESULTS = None


def _kernel_body(tc, io, stage=99):
    nc = tc.nc
    xT, WqT, WkT, WvE, WpT, BQ, BK, BMQ, ET, TRI, CMASK, Z = (
        io["xT"], io["WqT"], io["WkT"], io["WvE"], io["WpT"], io["BQ"],
        io["BK"], io["BMQ"], io["ET"], io["TRI"], io["CMASK"], io["Z"],
    )
    ONE8J, ZERO7 = io["ONE8J"], io["ZERO7"]

    from contextlib import ExitStack
    with ExitStack() as ctx:
        const = ctx.enter_context(tc.tile_pool(name="const", bufs=1))
        pmm = ctx.enter_context(tc.tile_pool(name="pmm", bufs=2, space="PSUM"))
        ps = ctx.enter_context(tc.tile_pool(name="ps", bufs=2, space="PSUM"))
        pot = ctx.enter_context(tc.tile_pool(name="pot", bufs=3, space="PSUM"))
        pbc = ctx.enter_context(tc.tile_pool(name="pbc", bufs=1, space="PSUM"))
        spt = ctx.enter_context(tc.tile_pool(name="spt", bufs=4))
        sden = ctx.enter_context(tc.tile_pool(name="sden", bufs=3))
        szout = ctx.enter_context(tc.tile_pool(name="szout", bufs=2))

        # ---- persistent SBUF tiles (unique tags in bufs=1 pool)
        def ctile(shape, tag, dt=F32R):
            return const.tile(shape, dt, tag=tag, name=tag)

        xt = [ctile([128, T], f"xt{i}") for i in range(4)]
        wq = [ctile([128, 256], f"wq{i}") for i in range(4)]
        wk = [ctile([128, 256], f"wk{i}") for i in range(4)]
        wv = [ctile([128, 260], f"wv{i}") for i in range(4)]
        wv5 = ctile([8, 260], "wv5")
        wp = [ctile([128, 512], f"wp{i}") for i in range(2)]
        bmq = [ctile([128, T], f"bmq{i}") for i in range(GH)]
        et = ctile([128, T], "et")
        tri = ctile([128, 128], "tri")
        cmask = ctile([128, 1], "cmask", F32)
        bq_t = ctile([128, 2], "bq_t", F32)
        bk_t = ctile([128, 2], "bk_t", F32)
        one8j = ctile([8, 128], "one8j")
        one8b = ctile([72, 128], "one8b")
        qt = [ctile([128, T], f"qt{i}") for i in range(2)]
        kt = [ctile([128, T], f"kt{i}") for i in range(2)]
        ve = [ctile([128, 260], f"ve{i}") for i in range(8)]
        yt = [ctile([128, T], f"yt{i}") for i in range(2)]

        # ---- loads: partition-chunked so each big tile spreads over several
        # HWDGE queues, critical-path tiles (x, Wq, Wk) first
        def chunked(dst, src, n):
            p = dst.shape[0] // n
            for c in range(n):
                nc.sync.dma_start(
                    out=dst[c * p:(c + 1) * p, :], in_=src[c * p:(c + 1) * p, :]
                )

        for i in range(4):
            chunked(xt[i], xT[i * 128:(i + 1) * 128, :], 4)
            chunked(wq[i], WqT[i * 128:(i + 1) * 128, :], 2)
            chunked(wk[i], WkT[i * 128:(i + 1) * 128, :], 2)
        for i in range(4):
            chunked(wv[i], WvE[i * 128:(i + 1) * 128, :], 2)
        nc.sync.dma_start(out=wv5, in_=WvE[512:520, :])
        for i in range(2):
            chunked(wp[i], WpT[i * 128:(i + 1) * 128, :], 2)
        for i in range(GH):
            chunked(bmq[i][0:64, :], BMQ[i], 2)
            chunked(bmq[i][64:128, :], BMQ[i], 2)
        nc.sync.dma_start(out=et[0:64, :], in_=ET[:, :])
        nc.sync.dma_start(out=et[64:128, :], in_=ET[:, :])
        nc.sync.dma_start(out=tri, in_=TRI[:, :])
        nc.sync.dma_start(out=cmask, in_=CMASK[:, :])
        nc.sync.dma_start(out=one8j, in_=ONE8J[:, :])
        nc.sync.dma_start(out=one8b[64:72, :], in_=ONE8J[:, :])
        for t in range(2):
            nc.sync.dma_start(out=bq_t[:, t:t + 1], in_=BQ[t])
            nc.sync.dma_start(out=bk_t[:, t:t + 1], in_=BK[t])

        # ---- Q^T / K^T projections: out (d x i), contraction over c
        for wt, bt, out_t in ((wq, bq_t, qt), (wk, bk_t, kt)):
            for dt in range(2):
                for ib in range(2):
                    mmp = pmm.tile([128, 512], F32, tag="mm", name="mmp")
                    for ct in range(4):
                        nc.tensor.matmul(
                            mmp,
                            wt[ct][:, dt * 128:(dt + 1) * 128],
                            xt[ct][:, ib * 512:(ib + 1) * 512],
                            start=(ct == 0), stop=(ct == 3),
                        )
                    nc.vector.tensor_scalar_add(
                        out_t[dt][:, ib * 512:(ib + 1) * 512], mmp, bt[:, dt:dt + 1]
                    )

        # ---- V_ext: out (j x 260) per 128-token tile; bias+ones via extra row
        for jt in range(8):
            vp = pmm.tile([128, 260], F32, tag="mm", name="vp")
            for ct in range(4):
                nc.tensor.matmul(
                    vp,
                    xt[ct][:, jt * 128:(jt + 1) * 128],
                    wv[ct],
                    start=(ct == 0), stop=False,
                )
            nc.tensor.matmul(vp, one8j, wv5, start=False, stop=True)
            nc.vector.tensor_copy(ve[jt], vp)

        if stage <= 1:
            zs1 = szout.tile([128, 512], F32, tag="z", name="zs1")
            nc.vector.tensor_copy(zs1, qt[0][:, 0:512])
            nc.sync.dma_start(out=Z[0:128, :], in_=zs1)
            return

        # ---- attention
        for blk in range(2):
            q0 = blk * 512
            for hp in range(GH):
                dt, off = hp // 2, (hp % 2) * 64
                otp = pot.tile([65, 512], F32, tag="ot", name="otp")
                njt = 4 * (blk + 1)
                for jt in range(njt):
                    m = jt - 4 * blk          # >=0: diagonal-crossing tile
                    c0 = 128 * m if m >= 0 else 0
                    sp = ps.tile([128, 512], F32, tag="s", name="sp")
                    has_bias = blk == 0
                    nc.tensor.matmul(
                        sp[:, c0:],
                        kt[dt][off:off + 64, jt * 128:(jt + 1) * 128],
                        qt[dt][off:off + 64, q0 + c0:q0 + 512],
                        start=True, stop=not has_bias,
                    )
                    if has_bias:
                        nc.tensor.matmul(
                            sp[:, c0:],
                            et[off:off + 64, jt * 128:(jt + 1) * 128],
                            bmq[hp][off:off + 64, c0:512],
                            start=False, stop=True,
                        )
                    pt = spt.tile([128, 512], F32R, tag="pt", name="pt")
                    nc.scalar.activation(
                        pt[:, c0:], sp[:, c0:], AFT.Exp, bias=cmask[:, 0:1]
                    )
                    if m >= 0:
                        nc.vector.tensor_mul(
                            pt[:, c0:c0 + 128], pt[:, c0:c0 + 128], tri
                        )
                    if stage <= 2:
                        if blk == 0 and hp == 0 and jt == 0:
                            zs2 = szout.tile([128, 512], F32, tag="z", name="zs2")
                            nc.vector.tensor_copy(zs2, pt)
                            nc.sync.dma_start(out=Z[0:128, :], in_=zs2)
                        continue
                    nc.tensor.matmul(
                        otp[:, c0:],
                        ve[jt][:, 65 * hp:65 * hp + 65],
                        pt[:, c0:],
                        start=(jt == 0), stop=(jt == njt - 1),
                    )
                if stage <= 2:
                    continue
                den = sden.tile([72, 512], F32R, tag="den", name="den")
                nc.sync.dma_start(out=den[65:72, :], in_=ZERO7[:, :])
                with nc.allow_low_precision(reason="softmax denominator"):
                    nc.vector.reciprocal(den[64:65, :], otp[64:65, :])
                bcp = pbc.tile([64, 512], F32, tag="bc", name="bcp")
                nc.tensor.matmul(
                    bcp, one8b[64:72, 0:64], den[64:72, :],
                    start=True, stop=True,
                )
                bcs = sden.tile([64, 512], F32R, tag="bcs", name="bcs")
                nc.vector.tensor_copy(bcs, bcp)
                nc.vector.tensor_mul(
                    yt[dt][off:off + 64, q0:q0 + 512], otp[0:64, :], bcs
                )

        if stage == 2:
            return
        if stage == 3:
            zs3 = szout.tile([128, 512], F32, tag="z", name="zs3")
            nc.vector.tensor_copy(zs3, yt[0][:, 0:512])
            nc.sync.dma_start(out=Z[0:128, :], in_=zs3)
            return

        # ---- partial projection Z = y^T.T @ WpT_g
        for it in range(8):
            zp = pmm.tile([128, 512], F32, tag="mm", name="zp")
            for ct in range(2):
                nc.tensor.matmul(
                    zp,
                    yt[ct][:, it * 128:(it + 1) * 128],
                    wp[ct],
                    start=(ct == 0), stop=(ct == 1),
                )
            zs = szout.tile([128, 512], F32, tag="z", name="zs")
            nc.vector.tensor_copy(zs, zp)
            nc.sync.dma_start(out=Z[it * 128:(it + 1) * 128, :], in_=zs)


def _build(stage=99):
    nc = bacc.Bacc("TRN2", target_bir_lowering=False, debug=False,
                   num_devices=N_CORES)
    io = {}

    def din(name, shape, dt=F32R):
        io[name] = nc.dram_tensor(name, shape, dt, kind="ExternalInput").ap()

    din("xT", (C, T))
    din("WqT", (C, 256))
    din("WkT", (C, 256))
    din("WvE", (520, 260))
    din("WpT", (256, C))
    din("BQ", (2, 128, 1), F32)
    din("BK", (2, 128, 1), F32)
    din("BMQ", (GH, 64, T))
    din("ET", (64, T))
    din("TRI", (128, 128))
    din("CMASK", (128, 1), F32)
    din("ONE8J", (8, 128))
    din("ZERO7", (7, 512))
    io["Z"] = nc.dram_tensor("Z", (T, C), F32, kind="ExternalOutput").ap()

    with tile.TileContext(nc) as tc:
        _kernel_body(tc, io, stage)
    nc.compile()
    return nc


def _one8j():
    a = np.zeros((8, 128), np.float32)
    a[0, :] = 1.0
    return a


def _host_prep(x, attn_bias, Wq, bq, Wk, bk, Wv, bv, Wp, bp):
    """Build the 8 per-core input maps."""
    f = np.float32
    ET = np.zeros((64, T), f)
    for gj in range(64):
        ET[gj, gj * 8:(gj + 1) * 8] = 1.0
    TRI = (np.arange(128)[None, :] >= np.arange(128)[:, None]).astype(f)
    CMASK = np.zeros((128, 1), f)
    CMASK[15::16] = -1e30

    in_maps = []
    for core in range(N_CORES):
        b, g = core // 2, core % 2
        gs = slice(256 * g, 256 * (g + 1))
        WqT = np.ascontiguousarray((Wq[gs, :] * SCALE).T, dtype=f)
        WkT = np.ascontiguousarray(Wk[gs, :].T, dtype=f)
        WvE = np.zeros((520, 260), f)
        for hp in range(GH):
            r = slice(256 * g + 64 * hp, 256 * g + 64 * hp + 64)
            WvE[:C, 65 * hp:65 * hp + 64] = Wv[r, :].T
            WvE[C, 65 * hp:65 * hp + 64] = bv[r]
            WvE[C, 65 * hp + 64] = 1.0
        WpT = np.ascontiguousarray(Wp[:, gs].T, dtype=f)
        BMQ = np.zeros((GH, 64, T), f)
        for hp in range(GH):
            h = GH * g + hp
            BMQ[hp, :, :512] = np.repeat(attn_bias[h], 8, axis=0).T
        in_maps.append({
            "ONE8J": _one8j(),
            "ZERO7": np.zeros((7, 512), f),
            "xT": np.ascontiguousarray(x[b].T, dtype=f),
            "WqT": WqT,
            "WkT": WkT,
            "WvE": WvE,
            "WpT": WpT,
            "BQ": np.ascontiguousarray((bq[gs] * SCALE).reshape(2, 128, 1), f),
            "BK": np.ascontiguousarray(bk[gs].reshape(2, 128, 1), f),
            "BMQ": BMQ,
            "ET": ET,
            "TRI": TRI,
            "CMASK": CMASK,
        })
    return in_maps


def kernel(**inputs):
    global LAST_RESULTS
    if "nc" not in _CACHE:
        _CACHE["nc"] = _build()
    nc = _CACHE["nc"]

    in_maps = _host_prep(**{k: np.asarray(v) for k, v in inputs.items()})
    res = run_bass_kernel_spmd(nc, in_maps, core_ids=list(range(N_CORES)))
    LAST_RESULTS = res

    bp = np.asarray(inputs["bp"], np.float32)
    out = np.empty((B, T, C), np.float32)
    for b in range(B):
        out[b] = (np.asarray(res.results[2 * b]["Z"])
                  + np.asarray(res.results[2 * b + 1]["Z"])
                  + bp[None, :])
    return out



# revision 97
# speedup vs baseline: 1.9306x; 1.9306x over previous
"""Trainium2 Bass kernel for CausalSelfAttention with kron-structured bias and
column-masked causal attention.

Shapes (hardcoded): x (4,1024,512), H=8 heads, HD=64, attn_bias (8,64,64)
expanded by kron(ones(8,8)) onto the top-left 512x512 of the (1024,1024)
score matrix. Causal tril mask with every 16th column (j%16==15) zeroed.

Sharding: 8 cores = 4 batches x 2 head-groups (4 heads each). Every core runs
an identical program on its own slice. Design notes (~79us HW, vs 143-190us
fp32 baseline):
  - everything bf16 on the wire and in SBUF (PSUM accumulates fp32); host
    packs per-core inputs into 7 contiguous bf16 blobs, DMA'd in waves
    (x+wq+wk concurrently from both HWDGE queues, then bias consts + wv,
    then wp) so compute starts while later waves stream in.
  - ~26 dummy matmuls on a zeroed tile warm the PE HAM clock-gate to
    2.4 GHz before the first real projection matmul arrives.
  - the PE queue executes in order, so matmul emission order IS the
    schedule: only the (dt0,kb0) Q/K projections are emitted up front; the
    other six groups, and each V-projection tile, are deferred into the
    attention loop right before first use (avoids head-of-line stalls on
    loads). NOTE: this emission order is a tuned local optimum - reordering
    den/AV, moving projections earlier, or masking on GpSimd all measured
    SLOWER (81-117us).
  - scores are computed per (blk, dt) head-PAIR: the two heads of a dt group
    sit in partition rows 0:64 / 64:128 of qt/kt, so their K=64 score matmuls
    land on disjoint PE row-groups (tile_position auto-derived) and run
    concurrently in the array.
  - the kron bias (key-group expansion via E^T) accumulates as a second K=64
    matmul pair; the causal triangular mask on the diagonal-crossing 128-col
    block accumulates as a third matmul (IDENT^T @ TRIL(-1e30)) - no DVE
    masking pass at all.
  - both heads' scores live in one 2-bank PSUM tile [128, 1024]; ONE ScalarE
    exp (bias = column-mask -1e30 per masked key partition) covers the pair.
  - V_ext carries a ones column per head so O^T row 64 accumulates the
    softmax denominator; 1/den via the single-instruction DVE
    reciprocal_approx_fast, broadcast to 64 partitions on the (idle) GpSimd
    engine, one DVE multiply normalizes.
  - the V bias bv is dropped on-device: softmax rows sum to 1, so its
    contribution is exactly Wp @ bv, folded into bp on the host.
  - partial projection Z = y^T.T @ Wp^T slice streamed out per 128-token
    tile in bf16; host sums the two per-batch partials and adds bp_eff.
"""

import sys
import types

import numpy as np

import concourse.bass as bass  # noqa: F401  (import keeps bass registered)
import concourse.bacc as bacc
import concourse.tile as tile
from concourse import mybir
from concourse.bass_utils import run_bass_kernel_spmd


def _ensure_axon_hooks():
    """bass_utils' trace path imports antenv.axon_hooks unconditionally; some
    images lack that module. Provide it (and register the real NTFF hook when
    the axon boot shim is available) so tracing degrades gracefully."""
    try:
        import antenv.axon_hooks  # noqa: F401
        return
    except ImportError:
        pass
    m = types.ModuleType("antenv.axon_hooks")
    m._hook = None
    m.set_axon_ntff_profile_hook = lambda h: setattr(m, "_hook", h)
    m.get_axon_ntff_profile_hook = lambda: m._hook
    sys.modules["antenv.axon_hooks"] = m
    try:
        import antenv
        antenv.axon_hooks = m
    except ImportError:
        pass
    try:
        from trn_agent_boot.trn_boot import _ntff_profile_via_ctypes
        m.set_axon_ntff_profile_hook(
            _ntff_profile_via_ctypes("/opt/axon/libaxon_pjrt.so")
        )
    except Exception:
        pass


_ensure_axon_hooks()

F32 = mybir.dt.float32
BF16 = mybir.dt.bfloat16
AFT = mybir.ActivationFunctionType
NPBF16 = mybir.dt.np(mybir.dt.bfloat16)

B, T, C, H = 4, 1024, 512, 8
HD = 64
SCALE = 1.0 / 8.0
GH = 4          # heads per core
N_CORES = 8
NEG = -1.0e30

# ---- packed blob column map (bf16, [128, NCOLS]) -----------------------
# per ct in 0..3: [wq_ct 256 | wk_ct 256 | x_ct 1024]  -> 4 * 1536
# then, in DMA-wave order: attention consts (bmq/et/tril/id), wv, wp.
CB = 1536
OFF_BMQ = 4 * CB           # 2 x 512
OFF_ET = OFF_BMQ + 1024    # 512
OFF_TRIL = OFF_ET + 512    # 128 (-1e30 strict-lower, keys x rel-queries)
OFF_ID = OFF_TRIL + 128    # 128
OFF_WV = OFF_ID + 128      # 4 x 256
OFF_WP = OFF_WV + 1024     # 2 x 512
NCOLS = OFF_WP + 1024      # 9984

_CACHE = {}
LAST_RESULTS = None


def _kernel_body(tc, io, stage=99):
    nc = tc.nc
    BIGS, CONSTF, Z = io["BIGS"], io["CONSTF"], io["Z"]

    from contextlib import ExitStack
    with ExitStack() as ctx:
        const = ctx.enter_context(tc.tile_pool(name="const", bufs=1))
        pmm = ctx.enter_context(tc.tile_pool(name="pmm", bufs=2, space="PSUM"))
        ps2 = ctx.enter_context(tc.tile_pool(name="ps2", bufs=2, space="PSUM"))
        pot = ctx.enter_context(tc.tile_pool(name="pot", bufs=1, space="PSUM"))
        spt = ctx.enter_context(tc.tile_pool(name="spt", bufs=3))
        srcp = ctx.enter_context(tc.tile_pool(name="srcp", bufs=2))
        szout = ctx.enter_context(tc.tile_pool(name="szout", bufs=2))

        def ctile(shape, tag, dt=BF16):
            return const.tile(shape, dt, tag=tag, name=tag)

        # ---- persistent SBUF tiles
        big = [ctile([128, CB], f"big{ct}") for ct in range(4)]      # wq|wk|x
        biga = ctile([128, 1792], "biga")                            # bmq|et|tril|id
        bigv = ctile([128, 1024], "bigv")                            # wv
        bigp = ctile([128, 1024], "bigp")                            # wp
        constf = ctile([128, 5], "constf", F32)
        qt = [[ctile([128, 512], f"qt{dt}{kb}") for kb in range(2)]
              for dt in range(2)]
        kt = [[ctile([128, 512], f"kt{dt}{kb}") for kb in range(2)]
              for dt in range(2)]
        ve = [ctile([128, 256], f"ve{j}") for j in range(8)]
        yt = [[ctile([128, 512], f"yt{blk}{dt}") for dt in range(2)]
              for blk in range(2)]
        onesc = ctile([128, 1], "onesc")
        scr = ctile([1, 16], "scr", F32)
        warm = ctile([128, 640], "warm")  # never written: PE warm-up fodder

        # slice helpers into the packed blob
        def wqs(ct, dt):
            return big[ct][:, dt * 128:dt * 128 + 128]

        def wks(ct, dt):
            return big[ct][:, 256 + dt * 128:256 + dt * 128 + 128]

        def xs(ct, t0, n):
            return big[ct][:, 512 + t0:512 + t0 + n]

        def wvs(ct):
            return bigv[:, ct * 256:ct * 256 + 256]

        def wps(ct):
            return bigp[:, ct * 512:ct * 512 + 512]

        def bmqs(dt):
            return biga[:, dt * 512:dt * 512 + 512]

        et2 = lambda: biga[:, 1024:1536]  # noqa: E731
        tril = lambda: biga[:, 1536:1664]  # noqa: E731
        ident = lambda: biga[:, 1664:1792]  # noqa: E731
        cmask = constf[:, 0:1]

        # ---- loads. Everything the attention pacer needs (x, wq/wk, bias
        # consts) goes up front, dispatched alternately from the Sync and
        # Scalar HWDGE queues so dispatch doesn't serialize; wv/wp (only
        # needed later) are chained behind so they don't steal bandwidth.
        # PE warm-up: ~6us of dummy matmuls so the HAM clock-gate reaches
        # 8/8 before the real projection matmuls arrive (they'd otherwise
        # all run at 1.2 GHz during the load phase).
        nc.vector.memset(warm, 0.0)
        wout = pmm.tile([128, 512], F32, tag="mm", name="wout")
        for _ in range(26):
            nc.tensor.matmul(wout, warm[:, 0:128], warm[:, 128:640],
                             start=True, stop=True)

        from concourse.tile_rust import add_dep_helper
        nc.sync.dma_start(out=constf, in_=CONSTF[:, :])
        w1 = []
        for ct in range(4):
            eng = nc.sync if ct % 2 == 0 else nc.scalar
            w1.append(eng.dma_start(out=big[ct], in_=BIGS[ct][:, :]))
        w2 = [nc.sync.dma_start(out=biga, in_=BIGS[4][:, :]),
              nc.scalar.dma_start(out=bigv, in_=BIGS[5][:, :])]
        for d in w2:
            for p in w1:
                add_dep_helper(d.ins, p.ins, True, "dma wave order")
        d4 = nc.sync.dma_start(out=bigp, in_=BIGS[6][:, :])
        for p in w2:
            add_dep_helper(d4.ins, p.ins, True, "dma wave order")

        # preload the exp table while DMAs run (first real exp then costs ~0)
        nc.scalar.activation(scr[0:1, 0:1], constf[0:1, 0:1], AFT.Exp)

        # ones column for the denominator matmuls
        nc.gpsimd.memset(onesc, 1.0)

        # ---- Q^T / K^T projections: out [128 dims(2 heads), 512 tokens].
        # Only the (dt0, kb0) pair is emitted up front (the minimum the
        # (blk0, dt0) attention needs); the other six groups are deferred
        # into the first attention loop so the in-order PE queue reaches the
        # first score matmuls ~10us earlier. Deferred groups do their bias
        # add on DVE to keep ScalarE free for the exp stream.
        proj_groups = []
        for dt, kb in ((0, 0), (1, 0), (0, 1), (1, 1)):
            for wsl, bcol, out_t in ((wqs, 1, qt), (wks, 3, kt)):
                proj_groups.append((dt, kb, wsl, bcol, out_t))

        def emit_proj(dt, kb, wsl, bcol, out_t, on_act):
            mmp = pmm.tile([128, 512], F32, tag="mm", name="mmp")
            for ct in range(4):
                nc.tensor.matmul(
                    mmp, wsl(ct, dt), xs(ct, kb * 512, 512),
                    start=(ct == 0), stop=(ct == 3),
                )
            if on_act:
                nc.scalar.activation(
                    out_t[dt][kb], mmp, AFT.Identity,
                    bias=constf[:, bcol + dt:bcol + dt + 1],
                )
            else:
                nc.vector.tensor_scalar_add(
                    out_t[dt][kb], mmp, constf[:, bcol + dt:bcol + dt + 1]
                )

        for g in proj_groups[:2]:
            emit_proj(*g, on_act=True)
        proj_left = proj_groups[2:]

        def emit_proj_some(n):
            while n > 0 and proj_left:
                emit_proj(*proj_left.pop(0), on_act=False)
                n -= 1

        def dump(row0, src):
            zd = szout.tile([128, 512], BF16, tag="z", name="zd")
            nc.vector.tensor_copy(zd[:src.shape[0], :src.shape[1]], src)
            nc.sync.dma_start(
                out=Z[row0:row0 + src.shape[0], :src.shape[1]],
                in_=zd[:src.shape[0], :src.shape[1]],
            )

        if stage == 1:
            emit_proj_some(99)
            dump(0, qt[0][0])
            dump(128, kt[0][0])
            return

        # ---- V: out [128 tokens, 256] per 128-token tile. Emission is
        # deferred into the attention loop (right before the first AV use)
        # so the in-order PE queue never stalls behind the bigv load.
        ve_done = [False] * 8

        def emit_v(jt):
            if ve_done[jt]:
                return
            ve_done[jt] = True
            vp = pmm.tile([128, 256], F32, tag="mm", name="vp")
            for ct in range(4):
                nc.tensor.matmul(
                    vp, xs(ct, jt * 128, 128), wvs(ct),
                    start=(ct == 0), stop=(ct == 3),
                )
            nc.vector.tensor_copy(ve[jt], vp)

        if stage == 2:
            emit_proj_some(99)
            for jt in range(8):
                emit_v(jt)
            dump(0, ve[0])
            dump(128, ve[7])
            return

        # ---- final projection per 128-token tile (emitted per blk: it 0-3
        # are interleaved into the blk1 loop so the tail only carries it 4-7)
        def emit_projz(it):
            pblk = it // 4
            zp = pmm.tile([128, 512], F32, tag="mm", name="zp")
            for ct in range(2):
                nc.tensor.matmul(
                    zp,
                    yt[pblk][ct][:, (it % 4) * 128:(it % 4) * 128 + 128],
                    wps(ct),
                    start=(ct == 0), stop=(ct == 1),
                )
            zs = szout.tile([128, 512], BF16, tag="z", name="zs")
            nc.scalar.activation(zs, zp, AFT.Copy, bias=0.0)
            nc.sync.dma_start(out=Z[it * 128:(it + 1) * 128, :], in_=zs)

        # ---- attention: per (blk, dt) head-pair.
        # Both heads' O^T share one PSUM tile (rows 0:64 / 64:128) so their
        # M=64 AV matmuls land on disjoint PE column-groups and run
        # concurrently; same for the two M=1 denominator-row matmuls
        # (rows 0 / 32 of den2). Only the FIRST matmul into each bank gets
        # start=True (it clears the whole bank's has_written bits).
        for blk in range(2):
            for dt in range(2):
                otab = pot.tile([128, 512], F32, tag="ot", name="otab")
                den2 = pot.tile([33, 512], F32, tag="dn", name="den2")
                njt = 4 * (blk + 1)
                for jt in range(njt):
                    m = jt - 4 * blk
                    c0 = 128 * m if m >= 0 else 0
                    sp2 = ps2.tile([128, 1024], F32, tag="s", name="sp2")
                    for sub in range(2):
                        off, h0 = sub * 64, sub * 512
                        last = (blk == 1 and m < 0)
                        nc.tensor.matmul(
                            sp2[:, h0 + c0:h0 + 512],
                            kt[dt][jt // 4][off:off + 64,
                                            (jt % 4) * 128:(jt % 4) * 128 + 128],
                            qt[dt][blk][off:off + 64, c0:512],
                            start=True, stop=last,
                        )
                    if blk == 0:
                        for sub in range(2):
                            off, h0 = sub * 64, sub * 512
                            nc.tensor.matmul(
                                sp2[:, h0 + c0:h0 + 512],
                                et2()[off:off + 64, jt * 128:jt * 128 + 128],
                                bmqs(dt)[off:off + 64, c0:512],
                                start=False, stop=(m < 0),
                            )
                    if m >= 0:
                        for sub in range(2):
                            h0 = sub * 512
                            nc.tensor.matmul(
                                sp2[:, h0 + c0:h0 + c0 + 128],
                                ident(), tril(),
                                start=False, stop=True,
                            )
                    emit_proj_some(2)
                    emit_v(jt)
                    pt2 = spt.tile([128, 1024], BF16, tag="pt", name="pt2")
                    if c0 >= 256:
                        # joint exp would waste c0 >= 256 columns on the
                        # dead [512:512+c0] strip; two ops beat that
                        nc.scalar.activation(
                            pt2[:, c0:512], sp2[:, c0:512], AFT.Exp,
                            bias=cmask,
                        )
                        nc.scalar.activation(
                            pt2[:, 512 + c0:1024], sp2[:, 512 + c0:1024],
                            AFT.Exp, bias=cmask,
                        )
                    else:
                        nc.scalar.activation(
                            pt2[:, c0:1024], sp2[:, c0:1024], AFT.Exp,
                            bias=cmask,
                        )
                    if stage == 3:
                        if blk == 0 and dt == 0 and jt == 0:
                            dump(0, sp2[:, 0:512])
                            dump(128, sp2[:, 512:1024])
                            dump(256, pt2[:, 0:512])
                            dump(384, pt2[:, 512:1024])
                        continue
                    for sub in range(2):
                        nc.tensor.matmul(
                            otab[64 * sub:64 * sub + 64, c0:],
                            ve[jt][:, 128 * dt + 64 * sub:128 * dt + 64 * sub + 64],
                            pt2[:, 512 * sub + c0:512 * sub + 512],
                            start=(jt == 0),
                            stop=(jt == njt - 1 and sub == 1),
                            skip_group_check=True,
                        )
                    for sub in range(2):
                        nc.tensor.matmul(
                            den2[32 * sub:32 * sub + 1, c0:],
                            onesc,
                            pt2[:, 512 * sub + c0:512 * sub + 512],
                            start=(jt == 0),
                            stop=(jt == njt - 1 and sub == 1),
                            skip_group_check=True,
                        )
                if stage == 3:
                    continue
                if stage == 4:
                    if blk == 0 and dt == 0:
                        dump(0, otab[:, :])
                        dump(128, den2[:, :])
                    continue
                # normalize. Copies free the PSUM banks for the next
                # head-pair quickly; the last iteration reads otab directly
                # (no next user - shorter tail). Then 1/den (DVE approx) ->
                # broadcast (GpSimd) -> multiply.
                last_it = (blk == 1 and dt == 1)
                if not last_it:
                    oc = srcp.tile([128, 512], F32, tag="oc", name="oc")
                    nc.vector.tensor_copy(oc, otab)
                else:
                    oc = otab
                dens = [srcp.tile([1, 512], F32, tag=f"den{s}", name="den")
                        for s in range(2)]
                nc.vector.tensor_copy(dens[0], den2[0:1, :])
                nc.vector.tensor_copy(dens[1], den2[32:33, :])
                rba = srcp.tile([64, 512], F32, tag="rba", name="rba")
                rbb = srcp.tile([128, 512], F32, tag="rbb", name="rbb")
                for sub, rb in ((0, rba), (1, rbb)):
                    rcp = srcp.tile([1, 512], F32, tag=f"rcp{sub}",
                                    name="rcp")
                    with nc.allow_low_precision(reason="softmax denominator"):
                        nc.vector.reciprocal_approx_fast(rcp, dens[sub])
                    nc.gpsimd.partition_broadcast(rb, rcp)
                    if stage == 5 and blk == 0 and dt == 0 and sub == 0:
                        dump(0, rcp)
                nc.vector.tensor_mul(yt[blk][dt][0:64, :], oc[0:64, :], rba)
                nc.vector.tensor_mul(
                    yt[blk][dt][64:128, :], oc[64:128, :], rbb[64:128, :]
                )
                if stage == 5 and blk == 0 and dt == 0:
                    dump(128, rba)

        if stage == 5:
            dump(256, yt[0][0])
            return
        if stage in (3, 4):
            return

        # ---- projection tiles, streamed out per it
        for it in range(8):
            emit_projz(it)


def _build(stage=99):
    nc = bacc.Bacc("TRN2", target_bir_lowering=False, debug=False,
                   num_devices=N_CORES)
    io = {}
    shapes = [1536, 1536, 1536, 1536, 1792, 1024, 1024]
    io["BIGS"] = [
        nc.dram_tensor(f"B{i}", (128, w), BF16, kind="ExternalInput").ap()
        for i, w in enumerate(shapes)
    ]
    io["CONSTF"] = nc.dram_tensor("CONSTF", (128, 5), F32,
                                  kind="ExternalInput").ap()
    io["Z"] = nc.dram_tensor("Z", (T, C), BF16, kind="ExternalOutput").ap()

    with tile.TileContext(nc) as tc:
        _kernel_body(tc, io, stage)
    nc.compile()
    return nc


def _host_prep(x, attn_bias, Wq, bq, Wk, bk, Wv, bv, Wp, bp):
    """Build the 8 per-core input maps (packed bf16 blob + small f32)."""
    f = np.float32

    tril_neg = np.where(
        np.arange(128)[None, :] < np.arange(128)[:, None], f(NEG), f(0.0)
    )
    ident = np.eye(128, dtype=f)
    et2 = np.zeros((128, 512), f)
    for gj in range(64):
        et2[gj, gj * 8:(gj + 1) * 8] = 1.0
        et2[64 + gj, gj * 8:(gj + 1) * 8] = 1.0
    cmask = np.zeros((128,), f)
    cmask[15::16] = NEG

    in_maps = []
    for core in range(N_CORES):
        b, g = core // 2, core % 2
        gs = slice(256 * g, 256 * (g + 1))
        xT = np.ascontiguousarray(x[b].T, dtype=f)          # (512, 1024)
        WqS = (Wq[gs, :] * SCALE).T.astype(f)               # (512, 256)
        WkS = Wk[gs, :].T.astype(f)
        WvS = Wv[gs, :].T.astype(f)                         # (512, 256)
        WpS = Wp[:, gs].T.astype(f)                         # (256, 512)

        blob = np.zeros((128, NCOLS), f)
        for ct in range(4):
            rs = slice(128 * ct, 128 * (ct + 1))
            base = ct * CB
            blob[:, base:base + 256] = WqS[rs, :]
            blob[:, base + 256:base + 512] = WkS[rs, :]
            blob[:, base + 512:base + 1536] = xT[rs, :]
            blob[:, OFF_WV + ct * 256:OFF_WV + (ct + 1) * 256] = WvS[rs, :]
        for ct in range(2):
            blob[:, OFF_WP + ct * 512:OFF_WP + (ct + 1) * 512] = \
                WpS[128 * ct:128 * (ct + 1), :]
        for dt in range(2):
            col = OFF_BMQ + dt * 512
            for sub in range(2):
                h = GH * g + 2 * dt + sub
                bm = np.repeat(attn_bias[h], 8, axis=0).T.astype(f)  # (64,512)
                blob[64 * sub:64 * sub + 64, col:col + 512] = bm
        blob[:, OFF_ET:OFF_ET + 512] = et2
        blob[:, OFF_TRIL:OFF_TRIL + 128] = tril_neg
        blob[:, OFF_ID:OFF_ID + 128] = ident

        constf = np.zeros((128, 5), f)
        constf[:, 0] = cmask
        constf[:, 1] = (bq[gs] * SCALE)[0:128]
        constf[:, 2] = (bq[gs] * SCALE)[128:256]
        constf[:, 3] = bk[gs][0:128]
        constf[:, 4] = bk[gs][128:256]

        bb = blob.astype(NPBF16)
        in_maps.append({
            "B0": np.ascontiguousarray(bb[:, 0:CB]),
            "B1": np.ascontiguousarray(bb[:, CB:2 * CB]),
            "B2": np.ascontiguousarray(bb[:, 2 * CB:3 * CB]),
            "B3": np.ascontiguousarray(bb[:, 3 * CB:4 * CB]),
            "B4": np.ascontiguousarray(bb[:, OFF_BMQ:OFF_BMQ + 1792]),
            "B5": np.ascontiguousarray(bb[:, OFF_WV:OFF_WV + 1024]),
            "B6": np.ascontiguousarray(bb[:, OFF_WP:NCOLS]),
            "CONSTF": constf,
        })
    return in_maps


def kernel(**inputs):
    global LAST_RESULTS
    if "nc" not in _CACHE:
        _CACHE["nc"] = _build()
    nc = _CACHE["nc"]

    np_inputs = {k: np.asarray(v) for k, v in inputs.items()}
    in_maps = _host_prep(**np_inputs)
    res = run_bass_kernel_spmd(nc, in_maps, core_ids=list(range(N_CORES)))
    LAST_RESULTS = res

    Wp = np.asarray(inputs["Wp"], np.float32)
    bv = np.asarray(inputs["bv"], np.float32)
    bp_eff = np.asarray(inputs["bp"], np.float32) + Wp @ bv
    out = np.empty((B, T, C), np.float32)
    for b in range(B):
        out[b] = (np.asarray(res.results[2 * b]["Z"]).astype(np.float32)
                  + np.asarray(res.results[2 * b + 1]["Z"]).astype(np.float32)
                  + bp_eff[None, :])
    return out
